# revision 1
# baseline (speedup 1.0000x reference)
"""Multi-head attention (B=2, S=2048, D=1024, H=16, causal) on 8 trn2 cores.

Sharding: core c -> batch b = c//4, head group g = c%4 (4 heads of 64 dims):
data parallel over batch, tensor/head parallel within it (W_q/W_k/W_v split
column-wise, W_o row-wise per head group).  Each core computes Q/K/V
projections for its head group over the full sequence, causal flash-style
attention, and the partial output projection A_g @ Wo.T[g_rows, :].  The host
pre-transposes activations/weight slices to fp16, sums the 4 output partials
per batch (the row-parallel unshard), and adds bo.

Device dataflow (per core, fp16 matmuls, fp32 accumulation/softmax):
  QT, KT   [256, S]  head-dim-major (transposed), computed by projection
           chains interleaved into the attention stream as PE gap fillers
  V_store  [S, 4*65] v columns + a ones column per head (so the AV matmul
           also produces the softmax denominators)
  scores^T tiles [128 ki, <=512 qi] -> exp on ACT (scale=1/sqrt(64)) -> E,
           diagonal tiles trimmed to their valid qi range; causal mask
           multiply only on the 128 straddling columns
  A^T      [256, S] accumulated in PSUM via (V|1)^T @ E, normalized by the
           ones-row sums (reciprocal_approx_fast + gpsimd partition
           broadcast + multiply)
  out      [S, 1024] fp32 partial = A_g @ WoT_g, projected in per-chunk
           bursts slipped into the next chunk's attention stream
"""

import ml_dtypes
import numpy as np

import concourse.bacc as bacc
import concourse.mybir as mybir
import concourse.tile as tile
from concourse.bass_utils import run_bass_kernel_spmd

F32 = mybir.dt.float32
BF16 = mybir.dt.float16  # fp16: same PE speed as bf16, 4x the mantissa
NP_BF16 = np.float16

S = 2048        # sequence length
E = 1024        # model dim (contraction for projections)
DG = 256        # head-group dim (4 heads x 64)
DH = 64         # head dim
NH = 4          # heads per core
ET = E // 128   # 8 e-tiles
ST = S // 128   # 16 s-tiles
SC = 512        # sequence chunk (psum free dim)
NSC = S // SC   # 4 chunks
SCALE = 1.0 / np.sqrt(DH)

_CACHED = {}


def _build():
    nc = bacc.Bacc("TRN2", target_bir_lowering=False, debug=False, num_devices=8)

    xqT = nc.dram_tensor("xqT", [E, S], BF16, kind="ExternalInput")
    xkT = nc.dram_tensor("xkT", [E, S], BF16, kind="ExternalInput")
    xvT = nc.dram_tensor("xvT", [E, S], BF16, kind="ExternalInput")
    wqT = nc.dram_tensor("wqT", [E, DG], BF16, kind="ExternalInput")
    wkT = nc.dram_tensor("wkT", [E, DG], BF16, kind="ExternalInput")
    wvT = nc.dram_tensor("wvT", [E, DG], BF16, kind="ExternalInput")
    woT = nc.dram_tensor("woT", [DG, E], BF16, kind="ExternalInput")
    bq = nc.dram_tensor("bq", [DG], F32, kind="ExternalInput")
    bk = nc.dram_tensor("bk", [DG], F32, kind="ExternalInput")
    bv = nc.dram_tensor("bv", [DG], BF16, kind="ExternalInput")
    out = nc.dram_tensor("out", [S, E], F32, kind="ExternalOutput")

    with tile.TileContext(nc) as tc:
        with (
            tc.tile_pool(name="persist", bufs=1) as pp,
            tc.tile_pool(name="xin", bufs=1) as xin,
            tc.tile_pool(name="epool", bufs=10) as epool,
            tc.tile_pool(name="opool", bufs=4) as opool,
            tc.tile_pool(name="small", bufs=2) as small,
            tc.tile_pool(name="ps_a", bufs=2, space="PSUM") as ps_a,
            tc.tile_pool(name="ps_e", bufs=2, space="PSUM") as ps_e,
            tc.tile_pool(name="ps_o", bufs=2, space="PSUM") as ps_o,
        ):
            # ---- input loads (weights first, then x: v, q, k) ----
            def load_x(name, dram, engs):
                tiles = [
                    xin.tile([128, S], BF16, tag=f"{name}{e}", name=f"{name}{e}")
                    for e in range(ET)
                ]
                for e in range(ET):
                    engs[e % len(engs)].dma_start(
                        tiles[e][:], dram.ap()[128 * e:128 * (e + 1), :])
                return tiles

            wq_sb = pp.tile([128, ET, DG], BF16, tag="wq")
            wk_sb = pp.tile([128, ET, DG], BF16, tag="wk")
            wv_sb = pp.tile([128, ET, DG], BF16, tag="wv")
            wo_sb = pp.tile([128, 2, E], BF16, tag="wo")
            bq_sb = pp.tile([128, 2], F32, tag="bq")
            bk_sb = pp.tile([128, 2], F32, tag="bk")
            bv_sb = pp.tile([1, DG], BF16, tag="bv")

            wv_view = wvT.ap().rearrange("(a p) d -> p a d", p=128)
            nc.sync.dma_start(wv_sb[:, 0:ET // 2, :], wv_view[:, 0:ET // 2, :])
            nc.scalar.dma_start(wv_sb[:, ET // 2:, :], wv_view[:, ET // 2:, :])
            nc.sync.dma_start(bv_sb[:], bv.ap().rearrange("(o d) -> o d", o=1))
            nc.gpsimd.dma_start(wq_sb[:], wqT.ap().rearrange("(a p) d -> p a d", p=128))
            nc.gpsimd.dma_start(wk_sb[:], wkT.ap().rearrange("(a p) d -> p a d", p=128))
            nc.gpsimd.dma_start(bq_sb[:], bq.ap().rearrange("(a p) -> p a", p=128))
            nc.gpsimd.dma_start(bk_sb[:], bk.ap().rearrange("(a p) -> p a", p=128))
            nc.gpsimd.dma_start(wo_sb[:], woT.ap().rearrange("(a p) j -> p a j", p=128))
            # xv in s-halves: V chains for the first 8 s-tiles start after 2MB
            xvh = [[None, None] for _ in range(ET)]
            for h in range(2):
                for e in range(ET):
                    t_ = xin.tile([128, S // 2], BF16, tag=f"xv{e}h{h}",
                                  name=f"xv{e}h{h}")
                    eng = [nc.sync, nc.scalar][(e + h) % 2]
                    eng.dma_start(
                        t_[:],
                        xvT.ap()[128 * e:128 * (e + 1),
                                 (S // 2) * h:(S // 2) * (h + 1)])
                    xvh[e][h] = t_
            xq_sb = load_x("xq", xqT, [nc.scalar, nc.sync])
            xk_sb = load_x("xk", xkT, [nc.sync, nc.scalar])

            ones_bf = pp.tile([1, 128], BF16, tag="ones_bf")
            nc.gpsimd.memset(ones_bf[:], 1.0)

            # causal strip: strip[p, f] = 1.0 if f - p >= 384 else 0.0
            strip = pp.tile([128, 896], BF16, tag="strip")
            nc.gpsimd.memset(strip[:], 1.0)
            nc.gpsimd.affine_select(
                out=strip[:],
                in_=strip[:],
                compare_op=mybir.AluOpType.is_ge,
                fill=0.0,
                base=-384,
                pattern=[[1, 896]],
                channel_multiplier=-1,
            )

            # ---- phase 1b first: V natural [S, 4*65] with ones columns ----
            vst = [pp.tile([128, NH * (DH + 1)], BF16, tag=f"vst{st}", name=f"vst{st}")
                   for st in range(ST)]
            for st in range(ST):
                nc.gpsimd.memset(vst[st][:], 1.0)
                ps = ps_a.tile([128, DG], F32, tag="ps_proj")
                nc.tensor.matmul(ps[:], ones_bf[:1, :], bv_sb[:1, :],
                                 start=True, stop=False)
                h, col = st // 8, 128 * (st % 8)
                for e in range(ET):
                    nc.tensor.matmul(
                        ps[:],
                        xvh[e][h][:, col:col + 128],
                        wv_sb[:, e, :],
                        start=False,
                        stop=(e == ET - 1),
                    )
                # single strided copy: psum [128, 4*64] -> vst cols {65h..65h+63}
                nc.vector.tensor_copy(
                    vst[st][:].rearrange("p (h x) -> p h x", h=NH)[:, :, 0:DH],
                    ps[:].rearrange("p (h x) -> p h x", h=NH),
                )

            # ---- QT/KT projection chains (emitted interleaved with attention) ----
            qt_sb = [pp.tile([128, S], BF16, tag=f"qt{d}", name=f"qt{d}") for d in range(2)]
            kt_sb = [pp.tile([128, S], BF16, tag=f"kt{d}", name=f"kt{d}") for d in range(2)]

            def proj_chain(x_sb, w_sb, b_sb, dst, sc, d):
                ps = ps_a.tile([128, SC], F32, tag="ps_proj",
                               name=f"pj{dst[0].name}{sc}{d}")
                for e in range(ET):
                    nc.tensor.matmul(
                        ps[:],
                        w_sb[:, e, 128 * d:128 * (d + 1)],
                        x_sb[e][:, SC * sc:SC * (sc + 1)],
                        start=(e == 0),
                        stop=(e == ET - 1),
                    )
                nc.vector.tensor_scalar_add(
                    dst[d][:, SC * sc:SC * (sc + 1)], ps[:], b_sb[:, d:d + 1]
                )

            def q_chain(sc, d):
                proj_chain(xq_sb, wq_sb, bq_sb, qt_sb, sc, d)

            def k_chain(sc, d):
                proj_chain(xk_sb, wk_sb, bk_sb, kt_sb, sc, d)

            # ---- phase 2 + 3: qc-major attention with output-projection bursts ----
            at_sb = [pp.tile([128, S], BF16, tag=f"at{d}", name=f"at{d}") for d in range(2)]

            def score_tile(pair, qc, t):
                """Both heads' score matmuls -> one 2-bank psum, single exp."""
                qt, kt = qt_sb[pair], kt_sb[pair]
                diag = t >= 4 * qc
                dd = 128 * t - SC * qc if diag else 0
                w = SC - dd
                pse = ps_e.tile([128, 2, SC], F32, tag="pse", name=f"pse{pair}{qc}{t}")
                for i in range(2):
                    p0 = 64 * i
                    nc.tensor.matmul(
                        pse[:, i, 0:w],
                        kt[p0:p0 + DH, 128 * t:128 * (t + 1)],
                        qt[p0:p0 + DH, SC * qc + dd:SC * (qc + 1)],
                        start=True, stop=True,
                    )
                e_sb = epool.tile([128, 2, SC], BF16, tag="esb",
                                  name=f"esb{pair}{qc}{t}")
                nc.scalar.activation(
                    e_sb[:, :, 0:w], pse[:, :, 0:w],
                    mybir.ActivationFunctionType.Exp,
                    bias=0.0, scale=float(SCALE),
                )
                if diag:
                    # only the first 128 trimmed columns straddle the triangle
                    for i in range(2):
                        nc.vector.tensor_mul(
                            e_sb[:, i, 0:128], e_sb[:, i, 0:128], strip[:, 384:512]
                        )
                return e_sb, dd, w

            def out_burst(qc, last=False, half=None):
                # output projection burst for finished qi rows of chunk qc
                sts = range(4 * qc, 4 * (qc + 1))
                if half is not None:
                    sts = sts[2 * half:2 * half + 2]
                for st in sts:
                    for jc in range(2):
                        ps = ps_a.tile([128, SC], F32, tag="ps_proj",
                                       name=f"psb3{st}{jc}")
                        for d in range(2):
                            nc.tensor.matmul(
                                ps[:],
                                at_sb[d][:, 128 * st:128 * (st + 1)],
                                wo_sb[:, d, SC * jc:SC * (jc + 1)],
                                start=(d == 0), stop=(d == 1),
                            )
                        o_sb = opool.tile([128, SC], F32, tag="osb",
                                          name=f"osb{st}{jc}")
                        if last and (st + jc) % 2 == 0:
                            nc.scalar.copy(o_sb[:], ps[:])
                        else:
                            nc.vector.tensor_copy(o_sb[:], ps[:])
                        eng = [nc.sync, nc.scalar, nc.gpsimd][(2 * st + jc) % 3] \
                            if last else nc.sync
                        eng.dma_start(
                            out.ap()[128 * st:128 * (st + 1), SC * jc:SC * (jc + 1)],
                            o_sb[:],
                        )

            # chunk-0 pair-0 projections must precede the attention stream
            q_chain(0, 0)
            k_chain(0, 0)
            for qc in range(NSC):
                nt = 4 * (qc + 1)  # ki tiles needed (causal)
                if qc == 0:
                    fillers = [lambda: q_chain(0, 1), lambda: k_chain(0, 1)]
                else:
                    fillers = []
                if qc + 1 < NSC:
                    for d in range(2):
                        fillers.append(lambda d=d: q_chain(qc + 1, d))
                        fillers.append(lambda d=d: k_chain(qc + 1, d))
                for pair in range(2):
                    psos = [ps_o.tile([128, SC], F32, tag="pso", name=f"pso{pair}{qc}{i}")
                            for i in range(2)]
                    # software pipeline: scores run two ki-tiles ahead of AV
                    es = {t0: score_tile(pair, qc, t0)
                          for t0 in range(min(2, nt))}
                    for t in range(nt):
                        if t + 2 < nt:
                            es[t + 2] = score_tile(pair, qc, t + 2)
                        if t in (2, 4) and pair == 0 and qc > 0:
                            # prev chunk's projection, split so the exp stream
                            # never drains during the burst
                            out_burst(qc - 1, half=t // 2 - 1)
                        elif t >= 1 and fillers:
                            fillers.pop(0)()  # next chunk's projection chain
                        e_sb, dd, w = es.pop(t)
                        for i in range(2):
                            nc.tensor.matmul(
                                psos[i][:65, dd:SC],
                                vst[t][:, 65 * (2 * pair + i):65 * (2 * pair + i + 1)],
                                e_sb[:, i, 0:w],
                                start=(t == 0), stop=(t == nt - 1),
                            )
                    # normalize by the ones-row sums (den/rec/bcast first so the
                    # gpsimd broadcasts overlap the au copies)
                    dens, recs, bcs, aus = [], [], [], []
                    for i in range(2):
                        den = small.tile([1, SC], F32, tag="den", bufs=4,
                                         name=f"den{pair}{qc}{i}")
                        nc.vector.tensor_copy(den[:], psos[i][64:65, :])
                        rec = small.tile([1, SC], F32, tag="rec", bufs=4,
                                         name=f"rec{pair}{qc}{i}")
                        nc.vector.reciprocal_approx_fast(rec[:], den[:])
                        bc = small.tile([64, SC], F32, tag="bc", bufs=4,
                                        name=f"bc{pair}{qc}{i}")
                        nc.gpsimd.partition_broadcast(bc[:], rec[:1, :])
                        bcs.append(bc)
                    for i in range(2):
                        au = small.tile([64, SC], F32, tag="au", bufs=4,
                                        name=f"au{pair}{qc}{i}")
                        nc.vector.tensor_copy(au[:], psos[i][:64, :])
                        aus.append(au)
                    for i in range(2):
                        nc.vector.tensor_mul(
                            at_sb[pair][64 * i:64 * i + DH, SC * qc:SC * (qc + 1)],
                            aus[i][:64, :],
                            bcs[i][:],
                        )
                assert not fillers, (qc, len(fillers))
            out_burst(NSC - 1, last=True)

    nc.compile()
    return nc


def _get_nc():
    if "nc" not in _CACHED:
        _CACHED["nc"] = _build()
    return _CACHED["nc"]


def _in_maps(q, k, v, Wq, bq, Wk, bk, Wv, bv, Wo, bo):
    B = q.shape[0]
    f32 = np.float32
    xT = {}
    for b in range(B):
        xT[("q", b)] = np.ascontiguousarray(q[b].T).astype(NP_BF16)
        xT[("k", b)] = np.ascontiguousarray(k[b].T).astype(NP_BF16)
        xT[("v", b)] = np.ascontiguousarray(v[b].T).astype(NP_BF16)
    maps = []
    for c in range(8):
        b, g = c // 4, c % 4
        rows = slice(DG * g, DG * (g + 1))
        maps.append({
            "xqT": xT[("q", b)],
            "xkT": xT[("k", b)],
            "xvT": xT[("v", b)],
            "wqT": np.ascontiguousarray(Wq[rows, :].T).astype(NP_BF16),
            "wkT": np.ascontiguousarray(Wk[rows, :].T).astype(NP_BF16),
            "wvT": np.ascontiguousarray(Wv[rows, :].T).astype(NP_BF16),
            "woT": np.ascontiguousarray(Wo[:, rows].T).astype(NP_BF16),
            "bq": np.ascontiguousarray(bq[rows], dtype=f32),
            "bk": np.ascontiguousarray(bk[rows], dtype=f32),
            "bv": np.ascontiguousarray(bv[rows]).astype(NP_BF16),
        })
    return maps


def _run(inputs, trace=False):
    nc = _get_nc()
    maps = _in_maps(
        inputs["q"], inputs["k"], inputs["v"],
        inputs["Wq"], inputs["bq"], inputs["Wk"], inputs["bk"],
        inputs["Wv"], inputs["bv"], inputs["Wo"], inputs["bo"],
    )
    res = run_bass_kernel_spmd(nc, maps, list(range(8)), trace=trace)
    parts = [r["out"] for r in res.results]
    bo_row = np.asarray(inputs["bo"], dtype=np.float32)
    out = np.stack([
        parts[0] + parts[1] + parts[2] + parts[3] + bo_row,
        parts[4] + parts[5] + parts[6] + parts[7] + bo_row,
    ]).astype(np.float32)
    return out, res


def kernel(**inputs):
    out, _ = _run(inputs, trace=False)
    return out

